# revision 4
# baseline (speedup 1.0000x reference)
"""GraphSAGE 2-layer GNN + MLP head on 8 Trainium2 NeuronCores (v6).

Strategy (dst-sharded, dense-adjacency scatter, fp8 DoubleRow, dst-slab
pipelined collectives):
  - Destination nodes sharded across 8 cores; node index space padded to
    1280 slots/core (10240 global slots = 80 full 128-chunks).
  - Scatter-mean collapses to  aggr = (relu(X W))^T A_mean  with
    A_mean[src,dst] = edge_count/deg(dst) in fp8 e4m3 (mean folded
    host-side), resident in SBUF and reused by both layers.
  - A is stored DST-SLAB-MAJOR in DRAM: three slabs of (512, 512, 226)
    dst columns, each slab [P, KC, ns] contiguous per partition. The
    layer-0 scatter completes full-K accumulation per slab as it
    streams, so early dst columns finish early and the inter-layer
    AllGather of y1 is split in 3 and pipelined: the ops chain back to
    back right after the framework's kernel-entry CC barrier (~48us),
    which is the hard floor for any cross-core data.
  - Layer-1 scatter runs in 3 waves as each gathered slab lands; psum
    banks (shared with layer 0) accumulate across waves, bank-staggered
    at the end so the per-slab tail (update + post_mp + log_softmax +
    output DMA) pipelines with the remaining banks' matmuls.
  - Sqrt/Exp/Ln activation tables preloaded at t=0 so no table load
    lands on the tail critical path.
"""

import numpy as np
import ml_dtypes

import concourse.bacc as bacc
import concourse.mybir as mybir
from concourse import tile
from concourse.bass_utils import run_bass_kernel_spmd

N_NODES = 10000
N_CORES = 8
SHARD = N_NODES // N_CORES   # 1250 real dst nodes per core
P = 128
JC = 10                      # local 128-chunks per core (1280 slots)
SLOTS = JC * P               # 1280 padded slots per core
G = N_CORES * SLOTS          # 10240 padded global slots
KC = G // P                  # 80 src chunks
KP = KC // 2                 # 40 DoubleRow pairs
F = 128
FOUT = 64
JSL = [(0, 4), (4, 8), (8, 10)]          # y1 chunk ranges per slab
NSL = [(0, 512), (512, 1024), (1024, 1250)]
NQUART = 4                   # a8 DMAs per slab (k-chunk quarters)

FP8 = mybir.dt.float8e4
BF16 = mybir.dt.bfloat16
F32 = mybir.dt.float32
DR = mybir.MatmulPerfMode.DoubleRow
AF = mybir.ActivationFunctionType

NP_FP8 = ml_dtypes.float8_e4m3
NP_BF16 = ml_dtypes.bfloat16


def _jc(j):
    """real node count in local chunk j (last chunk is partial: 98)."""
    return min(P, SHARD - j * P)


def build():
    nc = bacc.Bacc("TRN2", target_bir_lowering=False, debug=False,
                   num_devices=N_CORES)

    # ---- external I/O ----
    xt_d = nc.declare_dram_parameter("xt", [P, G], FP8, isOutput=False)
    xtsh_d = nc.declare_dram_parameter("xt_sh", [P, SLOTS], BF16, isOutput=False)
    a8_d = nc.declare_dram_parameter("a8", [P, KC * SHARD], FP8, isOutput=False)
    lin_w0_d = nc.declare_dram_parameter("lin_w0", [F, F], FP8, isOutput=False)
    lin_w1_d = nc.declare_dram_parameter("lin_w1", [F, F], BF16, isOutput=False)
    agg_w0_d = nc.declare_dram_parameter("agg_w0", [2 * F, F], BF16, isOutput=False)
    agg_w1_d = nc.declare_dram_parameter("agg_w1", [2 * F, F], BF16, isOutput=False)
    mp_w12_d = nc.declare_dram_parameter("mp_w12", [F, FOUT], BF16,
                                         isOutput=False)
    out_d = nc.declare_dram_parameter("out", [P, JC * FOUT], F32, isOutput=True)

    y1sh_d = []
    y1all_d = []
    for s, (j0, j1) in enumerate(JSL):
        cs = j1 - j0
        y1sh_d.append(nc.dram_tensor(f"y1sh{s}_d", [P, cs * F], FP8))
        y1all_d.append(nc.dram_tensor(f"y1all{s}_d", [N_CORES, P, cs * F], FP8,
                                      addr_space="Shared"))

    with tile.TileContext(nc) as tc:
        with (
            tc.tile_pool(name="persist", bufs=1) as pp,
            tc.tile_pool(name="work", bufs=2) as wp,
            tc.tile_pool(name="ps_s", bufs=1, space="PSUM") as ps_s,
            tc.tile_pool(name="ps_h", bufs=1, space="PSUM") as ps_h,
            tc.tile_pool(name="ps_b", bufs=1, space="PSUM") as ps_b,
            tc.tile_pool(name="ps_y", bufs=1, space="PSUM") as ps_y,
        ):
            # ---- persistent SBUF ----
            a_sb = pp.tile([P, KC * SHARD], FP8)          # dst-slab-major
            xt_sb = pp.tile([P, G], FP8)
            xtsh_sb = pp.tile([P, SLOTS], BF16)
            y_sb = pp.tile([P, KC, F], FP8)               # y0 (all nodes)
            y_l1 = pp.tile([P, KC, F], FP8)               # gathered y1
            y1loc = pp.tile([P, JC, F], FP8)
            x1T = pp.tile([P, SHARD], BF16)
            x2T = pp.tile([P, SHARD], BF16)
            zc = pp.tile([P, JC, FOUT], F32)
            expall = pp.tile([P, JC, FOUT], F32)
            outsb = pp.tile([P, JC, FOUT], F32)
            rmax = pp.tile([P, JC], F32)
            sumexp = pp.tile([P, JC], F32)
            lnsum = pp.tile([P, JC], F32)
            lin_w0_sb = pp.tile([F, F], FP8)
            lin_w1_sb = pp.tile([F, F], BF16)
            aggw0t_sb = pp.tile([F, F], BF16)
            aggw0b_sb = pp.tile([F, F], BF16)
            aggw1t_sb = pp.tile([F, F], BF16)
            aggw1b_sb = pp.tile([F, F], BF16)
            mp_w12_sb = pp.tile([F, FOUT], BF16)
            ones_mat = pp.tile([P, P], BF16)
            eps_sb = pp.tile([P, 1], F32)
            tblscr = pp.tile([P, 1], F32)

            # ---- a8 stream on the sync queue, starting immediately ----
            for s, (n0, n1) in enumerate(NSL):
                ns = n1 - n0
                off = KC * n0
                step = (KC // NQUART) * ns
                for q in range(NQUART):
                    nc.sync.dma_start(
                        a_sb[:, off + q * step: off + (q + 1) * step],
                        a8_d[:, off + q * step: off + (q + 1) * step])

            # ---- front loads on the scalar queue (parallel with a8) ----
            nc.scalar.dma_start(lin_w0_sb[:], lin_w0_d[:])
            XH = G // 2
            nc.scalar.dma_start(xt_sb[:, 0:XH], xt_d[:, 0:XH])
            nc.scalar.dma_start(xt_sb[:, XH:G], xt_d[:, XH:G])
            nc.scalar.dma_start(xtsh_sb[:], xtsh_d[:])
            nc.scalar.dma_start(aggw0t_sb[:], agg_w0_d[0:F, :])
            nc.scalar.dma_start(aggw0b_sb[:], agg_w0_d[F:2 * F, :])
            nc.scalar.dma_start(lin_w1_sb[:], lin_w1_d[:])
            nc.scalar.dma_start(aggw1t_sb[:], agg_w1_d[0:F, :])
            nc.scalar.dma_start(aggw1b_sb[:], agg_w1_d[F:2 * F, :])
            nc.scalar.dma_start(mp_w12_sb[:], mp_w12_d[:])

            nc.vector.memset(ones_mat[:], 1.0)
            nc.vector.memset(eps_sb[:], 1e-24)
            nc.vector.memset(y1loc[:, JC - 1, :], 0.0)
            nc.vector.memset(rmax[:], 0.0)
            nc.vector.memset(zc[:, :, :], 0.0)

            # preload Sqrt/Exp/Ln activation tables off the critical path
            nc.scalar.activation(tblscr[:], eps_sb[:], AF.Sqrt)
            nc.scalar.activation(tblscr[:], eps_sb[:], AF.Exp)
            nc.scalar.activation(tblscr[:], eps_sb[:], AF.Ln)

            def a_view(s):
                """[P, KC, ns] strided view of slab s inside flat a_sb."""
                n0, n1 = NSL[s]
                ns = n1 - n0
                off = KC * n0
                return a_sb[:, off:off + KC * ns].rearrange(
                    "p (k d) -> p k d", k=KC)

            # ---- y0 = relu(x @ w0) for ALL 80 chunks, 2-bank rotation
            for b in range(KC // 4):
                ps = ps_y.tile([P, 512], F32, tag=["ya", "yb"][b % 2],
                               name=f"y0_{b}")
                for q in range(4):
                    k = 4 * b + q
                    nc.tensor.matmul(ps[:, q * F:(q + 1) * F],
                                     xt_sb[:, k * P:(k + 1) * P],
                                     lin_w0_sb[:],
                                     start=True, stop=True,
                                     skip_group_check=True)
                nc.vector.tensor_scalar_max(
                    y_sb[:, 4 * b:4 * b + 4, :], ps[:], 0.0)

            ps_l1 = [ps_s.tile([P, 512], F32, tag=f"s{i}", name=f"ps_l1_{i}")
                     for i in range(3)]

            def sage_update(ps_bank, aggwt_sb, aggwb_sb, xin, xout, s):
                """concat-linear + relu + L2 row norm for dst slab s."""
                n0, n1 = NSL[s]
                ns = n1 - n0
                aggrT = wp.tile([P, 512], BF16, tag="aggrT")
                nc.vector.tensor_scalar_mul(aggrT[:, 0:ns], ps_bank[:, 0:ns],
                                            1.0)
                ph = ps_h.tile([P, 512], F32, tag="ph")
                nc.tensor.matmul(ph[:, 0:ns], aggwt_sb[:], xin[:, n0:n1],
                                 start=True, stop=False)
                nc.tensor.matmul(ph[:, 0:ns], aggwb_sb[:], aggrT[:, 0:ns],
                                 start=False, stop=True)
                hT = wp.tile([P, 512], F32, tag="hT")
                nc.vector.tensor_scalar_max(hT[:, 0:ns], ph[:, 0:ns], 0.0)
                h2 = wp.tile([P, 512], BF16, tag="h2")
                nc.vector.tensor_tensor(h2[:, 0:ns], hT[:, 0:ns],
                                        hT[:, 0:ns], mybir.AluOpType.mult)
                pb = ps_b.tile([P, 512], F32, tag="pb")
                nc.tensor.matmul(pb[:, 0:ns], ones_mat[:], h2[:, 0:ns],
                                 start=True, stop=True)
                nrm = wp.tile([P, 512], F32, tag="nrm")
                nc.scalar.activation(nrm[:, 0:ns], pb[:, 0:ns], AF.Sqrt,
                                     bias=eps_sb[:])
                rinv = wp.tile([P, 512], F32, tag="rinv")
                nc.vector.reciprocal_approx_fast(rinv[:, 0:ns], nrm[:, 0:ns])
                nc.vector.tensor_tensor(xout[:, n0:n1], hT[:, 0:ns],
                                        rinv[:, 0:ns], mybir.AluOpType.mult)

            def y1_slab(s):
                """y1 chunks for dst slab s: relu(x1 @ w1) into y1loc."""
                j0, j1 = JSL[s]
                ps = ps_y.tile([P, 512], F32, tag="ya", name=f"y1_{s}")
                for q, j in enumerate(range(j0, j1)):
                    jc = _jc(j)
                    nc.tensor.matmul(ps[0:jc, q * F:(q + 1) * F],
                                     x1T[:, j * P:j * P + jc], lin_w1_sb[:],
                                     start=True, stop=True,
                                     skip_group_check=True)
                nfull = sum(1 for j in range(j0, j1) if _jc(j) == P)
                if nfull:
                    nc.vector.tensor_scalar_max(
                        y1loc[:, j0:j0 + nfull, :],
                        ps[:, 0:nfull * F], 0.0)
                for q, j in enumerate(range(j0, j1)):
                    if _jc(j) < P:
                        nc.vector.tensor_scalar_max(
                            y1loc[0:_jc(j), j, :],
                            ps[0:_jc(j), q * F:(q + 1) * F], 0.0)

            yl1_v = y_l1[:, :, :].rearrange("p (c j) f -> p c (j f)",
                                            c=N_CORES)

            def reload(s):
                j0, j1 = JSL[s]
                nc.gpsimd.dma_start(yl1_v[:, :, j0 * F:j1 * F],
                                    y1all_d[s][:].transpose([1, 0, 2]))

            # ---- layer 0: scatter per dst slab as its a8 quarters land,
            # ---- then update + y1 + AllGather launch for that slab ----
            for s in range(3):
                n0, n1 = NSL[s]
                ns = n1 - n0
                av = a_view(s)
                for kp in range(KP):
                    nc.tensor.matmul(ps_l1[s][:, 0:ns],
                                     y_sb[:, 2 * kp:2 * kp + 2, :],
                                     av[:, 2 * kp:2 * kp + 2, 0:ns],
                                     start=(kp == 0), stop=(kp == KP - 1),
                                     perf_mode=DR)
                sage_update(ps_l1[s], aggw0t_sb, aggw0b_sb, xtsh_sb, x1T, s)
                y1_slab(s)
                j0, j1 = JSL[s]
                nc.scalar.dma_start(y1sh_d[s][:], y1loc[:, j0:j1, :])
                if s == 2:
                    # issue reload0 before trigger2 in gpsimd program order
                    reload(0)
                nc.gpsimd.collective_compute(
                    "AllGather", mybir.AluOpType.bypass,
                    replica_groups=[list(range(N_CORES))],
                    ins=[y1sh_d[s][:]], outs=[y1all_d[s][:]],
                )

            # ---- layer 1: 3 scatter waves, one per gathered slab; psum
            # ---- banks accumulate across waves (stop only at wave 2,
            # ---- bank-staggered); per-slab tail pipelined behind it ----
            wave_kps = []
            for s, (j0, j1) in enumerate(JSL):
                wave_kps.append([c * (JC // 2) + q for c in range(N_CORES)
                                 for q in range(j0 // 2, j1 // 2)])

            for s in range(3):
                if s > 0:
                    reload(s)
                if s < 2:
                    for kp in wave_kps[s]:
                        for i, (n0, n1) in enumerate(NSL):
                            nc.tensor.matmul(
                                ps_l1[i][:, 0:n1 - n0],
                                y_l1[:, 2 * kp:2 * kp + 2, :],
                                a_view(i)[:, 2 * kp:2 * kp + 2, 0:n1 - n0],
                                start=(s == 0 and kp == wave_kps[0][0]),
                                stop=False, perf_mode=DR)
                else:
                    # last wave: bank-major; per-slab tail follows each stop
                    for i, (n0, n1) in enumerate(NSL):
                        for kp in wave_kps[s]:
                            nc.tensor.matmul(
                                ps_l1[i][:, 0:n1 - n0],
                                y_l1[:, 2 * kp:2 * kp + 2, :],
                                a_view(i)[:, 2 * kp:2 * kp + 2, 0:n1 - n0],
                                start=False, stop=(kp == wave_kps[s][-1]),
                                perf_mode=DR)
                        sage_update(ps_l1[i], aggw1t_sb, aggw1b_sb, x1T,
                                    x2T, i)
                        # post_mp + log_softmax + output DMA for this slab
                        j0i, j1i = JSL[i]
                        pz = ps_y.tile([P, 512], F32, tag=["ya", "yb"][i % 2],
                                       name=f"ps_z{i}")
                        for q, j in enumerate(range(j0i, j1i)):
                            jc = _jc(j)
                            nc.tensor.matmul(
                                pz[0:jc, q * FOUT:(q + 1) * FOUT],
                                x2T[:, j * P:j * P + jc], mp_w12_sb[:],
                                start=True, stop=True,
                                skip_group_check=True)
                        nb = (j1i - j0i) * FOUT
                        pz3 = pz[:, 0:nb].rearrange("p (j f) -> p j f",
                                                    f=FOUT)
                        nc.vector.tensor_reduce(rmax[:, j0i:j1i], pz3,
                                                mybir.AxisListType.X,
                                                mybir.AluOpType.max)
                        nc.vector.tensor_tensor(
                            zc[:, j0i:j1i, :], pz3,
                            rmax[:, j0i:j1i].unsqueeze(2).broadcast_to(
                                [P, j1i - j0i, FOUT]),
                            mybir.AluOpType.subtract)
                        nc.scalar.activation(expall[:, j0i:j1i, :],
                                             zc[:, j0i:j1i, :], AF.Exp)
                        nc.vector.tensor_reduce(sumexp[:, j0i:j1i],
                                                expall[:, j0i:j1i, :],
                                                mybir.AxisListType.X,
                                                mybir.AluOpType.add)
                        nc.scalar.activation(lnsum[:, j0i:j1i],
                                             sumexp[:, j0i:j1i], AF.Ln)
                        nc.vector.tensor_tensor(
                            outsb[:, j0i:j1i, :], zc[:, j0i:j1i, :],
                            lnsum[:, j0i:j1i].unsqueeze(2).broadcast_to(
                                [P, j1i - j0i, FOUT]),
                            mybir.AluOpType.subtract)
                        nc.sync.dma_start(
                            out_d[:, j0i * FOUT:j1i * FOUT].rearrange(
                                "p (j f) -> p j f", f=FOUT),
                            outsb[:, j0i:j1i, :])

    nc.compile()
    return nc


_NC = None


def _get_nc():
    global _NC
    if _NC is None:
        _NC = build()
    return _NC


def make_in_maps(inputs):
    x = np.asarray(inputs["x"], dtype=np.float32)
    ei = np.asarray(inputs["edge_index"])
    src = ei[0].astype(np.int64)
    dst = ei[1].astype(np.int64)

    cnt = np.bincount(dst, minlength=N_NODES).astype(np.float32)
    inv = (1.0 / np.maximum(cnt, 1.0)).astype(np.float32)

    # dense scatter-mean matrix: edge_count/deg(dst), padded src slots,
    # partition-major per core
    srcp = (src // SHARD) * SLOTS + (src % SHARD)
    flat = srcp * N_NODES + dst
    counts = np.bincount(flat, minlength=G * N_NODES)
    A = counts.reshape(G, N_NODES).astype(np.float32)
    del counts
    A *= inv[None, :]
    A8 = A.astype(NP_FP8).reshape(KC, P, N_NODES).transpose(1, 0, 2)
    del A

    # padded transposed features [128, 10240]
    xp = np.zeros((G, F), np.float32)
    for c in range(N_CORES):
        xp[c * SLOTS:c * SLOTS + SHARD] = x[c * SHARD:(c + 1) * SHARD]
    xt8 = np.ascontiguousarray(xp.T).astype(NP_FP8)
    xt16 = np.ascontiguousarray(xp.T).astype(NP_BF16)

    def w(name, dt=NP_BF16):
        return np.ascontiguousarray(
            np.asarray(inputs[name], np.float32)).astype(dt)

    w12 = np.asarray(inputs["mp_w1"], np.float32) @ np.asarray(
        inputs["mp_w2"], np.float32)
    common = {
        "xt": xt8,
        "lin_w0": w("lin_w0", NP_FP8), "lin_w1": w("lin_w1"),
        "agg_w0": w("agg_w0"), "agg_w1": w("agg_w1"),
        "mp_w12": np.ascontiguousarray(w12).astype(NP_BF16),
    }
    in_maps = []
    for c in range(N_CORES):
        lo, hi = c * SHARD, (c + 1) * SHARD
        Ac = A8[:, :, lo:hi]  # [P, KC, SHARD]
        # dst-slab-major flatten: concat over slabs of [P, KC*ns]
        a8c = np.concatenate(
            [np.ascontiguousarray(Ac[:, :, n0:n1]).reshape(P, -1)
             for (n0, n1) in NSL], axis=1)
        in_maps.append({
            **common,
            "xt_sh": np.ascontiguousarray(xt16[:, c * SLOTS:(c + 1) * SLOTS]),
            "a8": np.ascontiguousarray(a8c),
        })
    return in_maps


def run(inputs, trace=False, **kwargs):
    nc = _get_nc()
    in_maps = make_in_maps(inputs)
    res = run_bass_kernel_spmd(nc, in_maps, core_ids=list(range(N_CORES)),
                               trace=trace, **kwargs)
    outs = []
    for c in range(N_CORES):
        o = res.results[c]["out"].reshape(P, JC, FOUT)
        outs.append(o.transpose(1, 0, 2).reshape(SLOTS, FOUT)[:SHARD])
    out = np.concatenate(outs, axis=0)
    return out.astype(np.float32), res


def kernel(**inputs):
    out, _ = run(inputs, trace=False)
    return out
